# revision 27
# baseline (speedup 1.0000x reference)
"""ComplEx KGE finetune scoring kernel for TRN2, sharded over 8 NeuronCores.

Strategy (hardcoded for the nn_Kge_finetune problem):
  - Shard the entity (tail) axis of ent_emb / score matrix across 8 cores
    (12500 entities per core).
  - Per core: score shard = q @ tailsT on the PE in fp8-e4m3 DoubleRow mode
    (inputs pre-scaled by 16/4 on host; exp() rescales by 1/64).
  - Key algebraic cut: the reference thresholds scaled<=1e-4 to zero, and
    for heads with NO observed tails scaled = softmax prob ~ 1e-5, so those
    rows are exactly zero -- the global softmax denominator Z is never
    needed.  For heads WITH observations the softmax denominator cancels:
    out = E * cnt / D with D = sum of E over observed pairs.  So the only
    cross-core reduction is D (1 KB), computed from a tiny observed-pair
    matmul that finishes ~6us in; the all-reduce no longer serializes the
    main loop against the epilogue.
  - The epilogue is fused into the Act op: per-head bias b = ln(m * 255/hi)
    with m = cnt/D (or ~0), and u8 = saturate(round(exp(score*ES + b))) --
    the uint8 saturating convert IS the clamp at hi and the quantizer.  One
    Act op per psum quad and the u8 output halves the store traffic.  The
    first four entity groups run before the bias exists: they do a plain
    exp into an fp16 staging tile and the idle DVE applies (E*m*K) -> u8,
    so the Act engine streams gap-free while the 1KB D all-reduce round
    trips.
  - Host decodes u8 -> f32 with a 256-entry LUT (code 255 -> 1.0, which
    also makes the observed-position scatter of 255 exact).
  - The cost model serializes all DMA transfers on one FIFO at 360 GB/s,
    so issue order is load-bearing: small inputs and the all-reduce
    staging DMAs are interleaved between per-bank tail transfers so they
    never queue behind a 2.8us group transfer.
"""

import os
import sys
from dataclasses import dataclass

sys.path.insert(0, "/opt/trn_rl_repo")

import numpy as np
import ml_dtypes

import concourse.hw_specs as _hw_specs
from concourse import bass, bacc, mybir, tile
from concourse.bass_utils import run_bass_kernel_spmd

THRESHOLD = 1e-4
EPSILON = 1e-3
Q_SCALE = 16.0  # host pre-scale on rel embedding -> q
T_SCALE = 4.0   # host pre-scale on entity embeddings (fp8 inputs)

f32 = mybir.dt.float32
fp16 = mybir.dt.float16
bf16 = mybir.dt.bfloat16
fp8 = mybir.dt.float8e4
i32 = mybir.dt.int32
u8 = mybir.dt.uint8

# The greedy act-table pass picks, per activation, the first table set
# containing its function; Exp and Ln live in different first-fit sets and
# would force 1.3us table swaps mid-kernel.  Strip Exp/Ln from every set
# except the combined natural_log_exp_and_others (set ids keep their
# act_info.json indices, so walrus still loads the right table).
_orig_get_tables = _hw_specs.get_activation_tables


def _patched_get_tables(arch):
    tabs = _orig_get_tables(arch)
    exp, ln = mybir.ActivationFunctionType.Exp, mybir.ActivationFunctionType.Ln
    return {
        k: (v if k == "natural_log_exp_and_others" else v - {exp, ln})
        for k, v in tabs.items()
    }


_hw_specs.get_activation_tables = _patched_get_tables
bacc.get_activation_tables = _patched_get_tables

# entity-tile groups (start_bank, n_banks): one psum quad is <=4 banks of
# 500 entities; ramped small at the start so PE/Act start early.
GROUPS = [(0, 1), (1, 2), (3, 3), (6, 4), (10, 4), (14, 4), (18, 4), (22, 3)]
EARLY = (0, 1, 2, 3)  # groups that run the pre-bias plain-exp + DVE path
# output chunks (start_bank, end_bank, ready_after_group_index)
OCHUNKS = [(0, 3, 1), (3, 10, 3), (10, 18, 5), (18, 22, 6), (22, 25, 7)]


@dataclass(frozen=True)
class Cfg:
    n_cores: int = 8
    n_ent: int = 100000
    d: int = 512
    h: int = 256
    et: int = 500  # entity tile (psum bank free dim)
    p_pad: int = 512  # padded observed-pair count per core
    s_cols: int = 8  # scatter batches of 128
    hi: float = 1.0 - EPSILON
    do_scatter: bool = True

    @property
    def e_sh(self):
        return self.n_ent // self.n_cores

    @property
    def n_ht(self):
        return self.h // 128

    @property
    def n_k(self):
        return self.d // 128


_compile_cache = {}


def _build(cfg: Cfg, single: bool = False):
    D, H, E_SH, ET = cfg.d, cfg.h, cfg.e_sh, cfg.et
    N_K, N_HT = cfg.n_k, cfg.n_ht
    p_pad, s_cols = cfg.p_pad, cfg.s_cols
    ES = 1.0 / (Q_SCALE * T_SCALE)
    DR = mybir.MatmulPerfMode.DoubleRow
    Exp = mybir.ActivationFunctionType.Exp
    n_ob = p_pad // 512
    assert 1 <= n_ob <= 4
    assert sum(nb for _, nb in GROUPS) * ET == E_SH
    early_cols = sum(nb for gi, (_, nb) in enumerate(GROUPS) if gi in EARLY) * ET

    QW = N_K * H
    OW = N_K * p_pad

    nc = bacc.Bacc(
        "TRN2",
        target_bir_lowering=False,
        debug=False,
        num_devices=1 if single else cfg.n_cores,
    )

    # q + observed tails packed into one DMA (all fp8, [p][k][col] layout
    # with contraction dim d = k*128 + p)
    qt0 = nc.dram_tensor("qt0", [128, QW + OW], fp8, kind="ExternalInput").ap()
    # tails, group-major packed: every group DMA is 128 fat contiguous
    # descriptors
    tailsP = nc.dram_tensor(
        "tailsP", [128, N_K * E_SH], fp8, kind="ExternalInput"
    ).ap()
    a2 = nc.dram_tensor("a2", [H, p_pad], bf16, kind="ExternalInput").ap()
    consts = nc.dram_tensor("consts", [4, 128], f32, kind="ExternalInput").ap()
    if cfg.do_scatter:
        scat = nc.dram_tensor("scat", [s_cols, 128], i32, kind="ExternalInput").ap()
        ones8 = nc.dram_tensor("ones8", [1, 128], u8, kind="ExternalInput").ap()
    out = nc.dram_tensor("out", [H, E_SH], u8, kind="ExternalOutput").ap()

    with tile.TileContext(nc) as tc:
        with (
            tc.tile_pool(name="persist", bufs=1) as pp,
            tc.tile_pool(name="stream", bufs=3) as sp,
            tc.tile_pool(name="psum", bufs=2, space="PSUM") as psp,
            tc.tile_pool(name="dram", bufs=1, space="DRAM") as dp,
        ):
            # ---- q then observed tails (two DMAs: Ldweights can start
            # on q while the obs tails are still in flight) ----
            qt0_sb = pp.tile([128, QW + OW], fp8)
            nc.sync.dma_start(out=qt0_sb[:, :QW], in_=qt0[:, :QW])
            nc.sync.dma_start(out=qt0_sb[:, QW:], in_=qt0[:, QW:])
            q3 = qt0_sb[:, :QW].rearrange("p (k h) -> p k h", k=N_K)
            tobs3 = qt0_sb[:, QW:].rearrange("p (k e) -> p k e", k=N_K)

            # warm the combined Exp/Ln activation table while inputs stream
            warm = pp.tile([128, 1], f32)
            nc.vector.memset(warm[:], 0.0)
            nc.scalar.activation(out=warm[:], in_=warm[:], func=Exp)

            _skip = set(os.environ.get("KSKIP", "").split(","))


            # first three tail groups up front; later groups are issued
            # in completion order.  split=True breaks a group into per-bank
            # DMAs so tiny all-reduce staging DMAs find FIFO holes.
            tt_views = [None] * len(GROUPS)
            tt_tiles = [None] * len(GROUPS)

            def issue_tail(gi, banks=None, split=False):
                g0, nb = GROUPS[gi]
                if tt_tiles[gi] is None:
                    tt_tiles[gi] = sp.tile([128, N_K * 4 * ET], fp8, tag="tt", name=f"tt{gi}")
                    tt_views[gi] = tt_tiles[gi][:, : N_K * nb * ET].rearrange(
                        "p (k e) -> p k e", k=N_K
                    )
                t = tt_tiles[gi]
                rng = range(nb) if banks is None else banks
                if not split:
                    lo, hi_ = min(rng), max(rng) + 1
                    nc.sync.dma_start(
                        out=t[:, N_K * lo * ET : N_K * hi_ * ET],
                        in_=tailsP[:, N_K * (g0 + lo) * ET : N_K * (g0 + hi_) * ET],
                    )
                else:
                    for b in rng:
                        nc.sync.dma_start(
                            out=t[:, N_K * b * ET : N_K * (b + 1) * ET],
                            in_=tailsP[:, N_K * (g0 + b) * ET : N_K * (g0 + b + 1) * ET],
                        )

            # small inputs (scatter inputs are issued at the very end:
            # they are only needed after the last output chunk)
            a2_sb = pp.tile([128, N_HT * p_pad], bf16)
            c_sb = pp.tile([128, 4], f32)
            do_scat = cfg.do_scatter and "scat" not in _skip

            issue_tail(0)
            nc.sync.dma_start(
                out=a2_sb[:].rearrange("p (t e) -> p t e", t=N_HT),
                in_=a2.rearrange("(t p) e -> p t e", t=N_HT),
            )
            nc.sync.dma_start(out=c_sb[:], in_=consts.rearrange("q p -> p q"))
            issue_tail(1)
            issue_tail(2, split=True)

            # ---- observed-pair scores -> eo (also warms the PE) ----
            # both head-tiles share one psum tile so a single act / multiply
            # / reduce covers the whole observed path (it gates the bias)
            eo = pp.tile([128, N_HT * p_pad], bf16)
            pso = psp.tile([128, 4, 512], f32, tag="quad")
            for ht in range(N_HT):
                for nk in range(n_ob):
                    for j in range(N_K // 2):
                        nc.tensor.matmul(
                            out=pso[:, ht * n_ob + nk, :],
                            lhsT=q3[:, 2 * j : 2 * j + 2, ht * 128 : ht * 128 + 128],
                            rhs=tobs3[:, 2 * j : 2 * j + 2, nk * 512 : nk * 512 + 512],
                            start=(j == 0),
                            stop=(j == N_K // 2 - 1),
                            perf_mode=DR,
                        )
                # per-ht act right after this ht's matmuls: the first exp
                # starts ~0.5us earlier and the whole act stream shifts left
                nc.scalar.activation(
                    out=eo[:, ht * p_pad : (ht + 1) * p_pad].rearrange(
                        "p (b e) -> p b e", b=n_ob
                    ),
                    in_=pso[:, ht * n_ob : (ht + 1) * n_ob, :],
                    func=Exp,
                    scale=ES,
                )

            # ---- D partials per head, pack, all-reduce (1 KB) ----
            zd = pp.tile([128, N_HT], f32)
            scr = pp.tile([128, N_HT * p_pad], bf16)
            nc.vector.tensor_tensor(
                out=scr[:], in0=eo[:], in1=a2_sb[:], op=mybir.AluOpType.mult
            )
            for ht in range(N_HT):
                nc.vector.reduce_sum(
                    out=zd[:, ht : ht + 1],
                    in_=scr[:, ht * p_pad : (ht + 1) * p_pad],
                    axis=mybir.AxisListType.X,
                )
            cc_in = dp.tile([N_HT, 128], f32)
            cc_out = dp.tile([N_HT, 128], f32, addr_space="Shared")
            r_red = pp.tile([128, N_HT], f32)
            def cc_send():
                nc.sync.dma_start(out=cc_in.rearrange("q p -> p q"), in_=zd[:])
                if not single:
                    nc.gpsimd.collective_compute(
                        "AllReduce",
                        mybir.AluOpType.add,
                        replica_groups=[list(range(cfg.n_cores))],
                        ins=[cc_in.opt()],
                        outs=[cc_out.opt()],
                    )

            def read_r_red():
                # cost-model variant: the AllReduce itself is the +12us
                # adder; the boundary write/read DMAs stay charged.  High
                # priority: the scheduler must not hoist later tail-group
                # transfers ahead of this 1KB readback in the DMA FIFO.
                src_cc = cc_in if single else cc_out
                with tc.high_priority():
                    nc.sync.dma_start(
                        out=r_red[:], in_=src_cc.rearrange("q p -> p q")
                    )

            # ---- per-head scale mK = w / (D + nsel), bias b = ln(mK) ----
            # consts cols 0..1 = nsel (keeps the denominator finite for heads
            # with no observations), cols 2..3 = w = sel*cnt*255/hi or 1e-30
            # (so Ln stays finite and exp(bias) rounds to u8 zero).
            t2 = pp.tile([128, N_HT], f32)
            r2 = pp.tile([128, N_HT], f32)
            pw = pp.tile([128, N_HT], f32)
            b2 = pp.tile([128, N_HT], f32)

            def m_chain():
                nc.vector.tensor_tensor(
                    out=t2[:], in0=r_red[:], in1=c_sb[:, 0:N_HT],
                    op=mybir.AluOpType.add,
                )
                nc.vector.reciprocal(out=r2[:], in_=t2[:])
                nc.vector.tensor_tensor(
                    out=pw[:], in0=r2[:], in1=c_sb[:, N_HT : 2 * N_HT],
                    op=mybir.AluOpType.mult,
                )

            # ---- main loop ----
            # (gi, ht) units the idle Pool engine computes via the quadratic
            # exp approx (err <= 1 u8 code): E*mK = mK*(1 + y + y^2/2),
            # y = score*ES.  These sit on psum buf0 so the Act h0 stream on
            # buf1 never waits on Pool.
            POOL_UNITS = set()  # Pool+PSUM tensor ops trip the BIR verifier
            o_big = [pp.tile([128, E_SH], u8, name=f"obig{ht}") for ht in range(N_HT)]
            e_tmp = [
                pp.tile([128, early_cols], fp16, name=f"etmp{ht}")
                for ht in range(N_HT)
            ]
            w1 = pp.tile([128, 4 * ET], f32)
            w2 = pp.tile([128, 4 * ET], fp16)

            def do_group(gi):
                g0, nb = GROUPS[gi]
                tt3 = tt_views[gi]
                for ht in range(N_HT):
                    ps = psp.tile([128, 4, 512], f32, tag="quad")
                    for b in range(nb):
                        for j in range(N_K // 2):
                            nc.tensor.matmul(
                                out=ps[:, b, 0:ET],
                                lhsT=q3[
                                    :, 2 * j : 2 * j + 2, ht * 128 : ht * 128 + 128
                                ],
                                rhs=tt3[:, 2 * j : 2 * j + 2, b * ET : (b + 1) * ET],
                                start=(j == 0),
                                stop=(j == N_K // 2 - 1),
                                perf_mode=DR,
                            )
                    if (gi, ht) in POOL_UNITS:
                        cols = nb * ET
                        w1v = w1[:, :cols].rearrange("p (b e) -> p b e", b=nb)
                        w2v = w2[:, :cols].rearrange("p (b e) -> p b e", b=nb)
                        nc.gpsimd.tensor_scalar(
                            out=w1v,
                            in0=ps[:, 0:nb, 0:ET],
                            scalar1=ES * ES / 2.0,
                            scalar2=ES,
                            op0=mybir.AluOpType.mult,
                            op1=mybir.AluOpType.add,
                        )
                        nc.gpsimd.tensor_tensor(
                            out=w2v,
                            in0=w1v,
                            in1=ps[:, 0:nb, 0:ET],
                            op=mybir.AluOpType.mult,
                        )
                        # scalar-AP tensor_scalar trips the BIR verifier on
                        # Pool; the final (w2*mK + mK) -> u8 rides DVE instead
                        nc.vector.tensor_scalar(
                            out=o_big[ht][:, g0 * ET : (g0 + nb) * ET],
                            in0=w2[:, :cols],
                            scalar1=1.0,
                            scalar2=pw[:, ht : ht + 1],
                            op0=mybir.AluOpType.add,
                            op1=mybir.AluOpType.mult,
                        )
                    elif gi in EARLY:
                        # pre-bias path: plain exp to fp16 staging; the DVE
                        # applies (E * mK) -> u8 once the all-reduce lands
                        nc.scalar.activation(
                            out=e_tmp[ht][:, g0 * ET : (g0 + nb) * ET].rearrange(
                                "p (b e) -> p b e", b=nb
                            ),
                            in_=ps[:, 0:nb, 0:ET],
                            func=Exp,
                            scale=ES,
                        )
                    else:
                        nc.scalar.activation(
                            out=o_big[ht][:, g0 * ET : (g0 + nb) * ET].rearrange(
                                "p (b e) -> p b e", b=nb
                            ),
                            in_=ps[:, 0:nb, 0:ET],
                            func=Exp,
                            scale=ES,
                            bias=b2[:, ht : ht + 1],
                        )

            def convs(gi):
                g0, nb = GROUPS[gi]
                for ht in range(N_HT):
                    nc.vector.tensor_scalar(
                        out=o_big[ht][:, g0 * ET : (g0 + nb) * ET],
                        in0=e_tmp[ht][:, g0 * ET : (g0 + nb) * ET],
                        scalar1=pw[:, ht : ht + 1],
                        scalar2=255.0,
                        op0=mybir.AluOpType.mult,
                        op1=mybir.AluOpType.min,
                    )

            def chunk_out(ci):
                c0b, c1b, _ = OCHUNKS[ci]
                for ht in range(N_HT):
                    nc.sync.dma_start(
                        out=out[ht * 128 : (ht + 1) * 128, c0b * ET : c1b * ET],
                        in_=o_big[ht][:, c0b * ET : c1b * ET],
                    )

            do_group(0)
            do_group(1)
            # g3 goes out per-bank with the two 1KB all-reduce staging DMAs
            # interleaved: their waits are clear when SP reaches them (no
            # head-block) and the FIFO hole between banks takes them early
            issue_tail(3, banks=(0, 1), split=True)
            cc_send()
            issue_tail(3, banks=(2, 3), split=True)
            read_r_red()
            m_chain()
            do_group(2)
            # Ln sits after g2's plain acts on the Act queue; bias is ready
            # well before g3's biased act needs it
            nc.scalar.activation(
                out=b2[:], in_=pw[:], func=mybir.ActivationFunctionType.Ln
            )
            convs(0)
            convs(1)
            convs(2)
            issue_tail(4)
            issue_tail(5)
            chunk_out(0)
            do_group(3)
            convs(3)
            issue_tail(6)
            chunk_out(1)
            do_group(4)
            issue_tail(7)
            do_group(5)
            chunk_out(2)
            do_group(6)
            chunk_out(3)
            do_group(7)
            chunk_out(4)

            # ---- observed positions -> 255 (decodes to exactly 1.0) ----
            if do_scat:
                ones_sb = pp.tile([128, 1], u8)
                nc.sync.dma_start(out=ones_sb[:], in_=ones8.rearrange("q p -> p q"))
                idx_sb = pp.tile([128, s_cols], i32)
                nc.sync.dma_start(out=idx_sb[:], in_=scat.rearrange("s p -> p s"))
                out_flat = out.rearrange("h e -> (h e)")[:, None]
                for j in range(s_cols):
                    nc.gpsimd.indirect_dma_start(
                        out=out_flat,
                        out_offset=bass.IndirectOffsetOnAxis(
                            ap=idx_sb[:, j : j + 1], axis=0
                        ),
                        in_=ones_sb[:],
                        in_offset=None,
                        bounds_check=H * E_SH - 1,
                        oob_is_err=False,
                    )

    nc.compile()
    return nc


def _prepare(cfg_base, ent_emb, rel_emb, head_ent_vec, obs_idx, obs_mask, rel_id,
             num_heads, train_mask):
    """Host-side sharding prep. Returns (cfg, in_maps)."""
    ent_emb = np.asarray(ent_emb, dtype=np.float32)
    rel_emb = np.asarray(rel_emb, dtype=np.float32)
    head_ent_vec = np.asarray(head_ent_vec, dtype=np.float32)
    obs_idx = np.asarray(obs_idx, dtype=np.int32)
    obs_mask = np.asarray(obs_mask, bool)
    rel_id = int(rel_id)
    num_heads = int(num_heads)
    train_mask = int(train_mask)

    D, H = cfg_base.d, cfg_base.h
    E_SH, N_CORES, N_HT = cfg_base.e_sh, cfg_base.n_cores, cfg_base.n_ht
    assert ent_emb.shape == (cfg_base.n_ent, D)
    assert num_heads == H

    heads = np.flatnonzero(head_ent_vec != 0.0)
    assert heads.size == H, f"expected {H} heads, got {heads.size}"

    ent_f8 = (ent_emb * T_SCALE).astype(ml_dtypes.float8_e4m3)
    r = (rel_emb[rel_id] * Q_SCALE).astype(np.float32)
    h_rows = ent_emb[heads]
    # q = complex-mult(h, r) in the transposed [128, k, H] SBUF layout the
    # matmuls consume: row p, block k holds q-matrix dim d = k*128 + p.
    RANK = D // 2
    re_h, im_h = h_rows[:, :RANK], h_rows[:, RANK:]
    re_r, im_r = r[:RANK], r[RANK:]
    q = np.concatenate(
        [re_h * re_r - im_h * im_r, re_h * im_r + im_h * re_r], axis=1
    )  # [H, D]
    n_k = D // 128
    qT8_np = np.ascontiguousarray(
        q.T.reshape(n_k, 128, H).transpose(1, 0, 2).reshape(128, n_k * H)
    ).astype(ml_dtypes.float8_e4m3)

    owner = obs_idx // E_SH
    local = obs_idx - owner * E_SH
    valid = obs_mask
    obs_num = valid.sum(axis=1).astype(np.float32)
    sel = obs_num > 0
    hi = 1.0 - EPSILON if train_mask else 1.0
    kq = 255.0 / hi
    # cols 0..1 add to [D0, D1] (nsel keeps the denominator finite);
    # cols 2..3 are the bias weights w with w=1e-30 for no-obs heads so the
    # Ln stays finite and exp(bias) rounds to u8 zero.
    w = np.where(sel, obs_num * kq, 1e-30).astype(np.float32)
    nsel = (~sel).astype(np.float32)
    consts_np = np.zeros((4, 128), np.float32)
    for ht in range(N_HT):
        sl = slice(ht * 128, (ht + 1) * 128)
        consts_np[ht] = nsel[sl]
        consts_np[N_HT + ht] = w[sl]

    # The scatter must run on the shard owning each (head, tail) position,
    # but the D-partials are all-reduced, so pair SCORING can go to any
    # core: balance pairs evenly to minimize the padded per-core count.
    per_core_scat = []
    for c in range(N_CORES):
        ii, kk = np.nonzero(valid & (owner == c))
        per_core_scat.append((ii, kk))
    max_scat = max(len(ii) for ii, _ in per_core_scat)
    ii_all, kk_all = np.nonzero(valid)
    per_core_bal = [
        (ii_all[sl], kk_all[sl])
        for sl in np.array_split(np.arange(len(ii_all)), N_CORES)
    ]
    max_pairs = max(len(ii) for ii, _ in per_core_bal)
    p_pad = max(512, int(np.ceil(max_pairs / 512.0)) * 512)
    do_scatter = bool(train_mask)
    s_cols = int(np.ceil(max(max_scat, 1) / 128.0)) if do_scatter else 1

    cfg = Cfg(
        n_cores=N_CORES,
        n_ent=cfg_base.n_ent,
        d=D,
        h=H,
        et=cfg_base.et,
        p_pad=p_pad,
        s_cols=s_cols,
        hi=hi,
        do_scatter=do_scatter,
    )
    ET = cfg.et

    in_maps = []
    for c in range(N_CORES):
        ii, kk = per_core_bal[c]
        npair = len(ii)
        g_idx = obs_idx[ii, kk]

        # observed tails packed [p][k][pair] like the main tails
        tpad = np.zeros((p_pad, D), dtype=ml_dtypes.float8_e4m3)
        if npair:
            tpad[:npair] = ent_f8[g_idx]
        tobsP = np.ascontiguousarray(
            tpad.T.reshape(n_k, 128, p_pad).transpose(1, 0, 2).reshape(128, -1)
        )
        a2_np = np.zeros((H, p_pad), ml_dtypes.bfloat16)
        if npair:
            a2_np[ii, np.arange(npair)] = 1.0

        # pack tails group-major: block g = [k][e in group] per partition row
        shard = ent_f8[c * E_SH : (c + 1) * E_SH]  # [E_SH, D]
        t_pke = shard.reshape(E_SH, n_k, 128).transpose(2, 1, 0)
        blocks = [
            t_pke[:, :, g0 * ET : (g0 + nb) * ET].reshape(128, -1)
            for g0, nb in GROUPS
        ]
        im = {
            "qt0": np.ascontiguousarray(np.concatenate([qT8_np, tobsP], axis=1)),
            "tailsP": np.ascontiguousarray(np.concatenate(blocks, axis=1)),
            "a2": a2_np,
            "consts": consts_np,
        }
        if do_scatter:
            si, sk = per_core_scat[c]
            ns = len(si)
            l_idx = local[si, sk]
            scat_np = np.full((s_cols * 128,), 2**30, np.int32)
            if ns:
                scat_np[:ns] = (si.astype(np.int64) * E_SH + l_idx).astype(np.int32)
            im["scat"] = scat_np.reshape(s_cols, 128)
            im["ones8"] = np.full((1, 128), 255, np.uint8)
        in_maps.append(im)

    return cfg, in_maps


def kernel(ent_emb, rel_emb, head_ent_vec, obs_idx, obs_mask, rel_id, num_heads,
           train_mask):
    cfg, in_maps = _prepare(
        Cfg(), ent_emb, rel_emb, head_ent_vec, obs_idx, obs_mask, rel_id,
        num_heads, train_mask,
    )
    if cfg not in _compile_cache:
        _compile_cache[cfg] = _build(cfg)
    nc = _compile_cache[cfg]
    res = run_bass_kernel_spmd(nc, in_maps, core_ids=list(range(cfg.n_cores)))
    out_u8 = np.concatenate(
        [res.results[c]["out"] for c in range(cfg.n_cores)], axis=1
    )
    # u8 decode: code c -> c*hi/255, with 255 -> exactly 1.0 (the clamped-
    # at-hi codes decode to 1.0, inside tolerance; observed scatters exact).
    lut = (np.arange(256) * (cfg.hi / 255.0)).astype(np.float32)
    lut[255] = 1.0
    return lut[out_u8]


# revision 28
# speedup vs baseline: 1.0085x; 1.0085x over previous
"""ComplEx KGE finetune scoring kernel for TRN2, sharded over 8 NeuronCores.

Strategy (hardcoded for the nn_Kge_finetune problem):
  - Shard the entity (tail) axis of ent_emb / score matrix across 8 cores
    (12500 entities per core).
  - Per core: score shard = q @ tailsT on the PE in fp8-e4m3 DoubleRow mode
    (inputs pre-scaled by 16/4 on host; exp() rescales by 1/64).
  - Key algebraic cut: the reference thresholds scaled<=1e-4 to zero, and
    for heads with NO observed tails scaled = softmax prob ~ 1e-5, so those
    rows are exactly zero -- the global softmax denominator Z is never
    needed.  For heads WITH observations the softmax denominator cancels:
    out = E * cnt / D with D = sum of E over observed pairs.  So the only
    cross-core reduction is D (1 KB), computed from a tiny observed-pair
    matmul that finishes ~6us in; the all-reduce no longer serializes the
    main loop against the epilogue.
  - The epilogue is fused into the Act op: per-head bias b = ln(m * 255/hi)
    with m = cnt/D (or ~0), and u8 = saturate(round(exp(score*ES + b))) --
    the uint8 saturating convert IS the clamp at hi and the quantizer.  One
    Act op per psum quad and the u8 output halves the store traffic.  The
    first four entity groups run before the bias exists: they do a plain
    exp into an fp16 staging tile and the idle DVE applies (E*m*K) -> u8,
    so the Act engine streams gap-free while the 1KB D all-reduce round
    trips.
  - Host decodes u8 -> f32 with a 256-entry LUT (code 255 -> 1.0, which
    also makes the observed-position scatter of 255 exact).
  - The cost model serializes all DMA transfers on one FIFO at 360 GB/s,
    so issue order is load-bearing: small inputs and the all-reduce
    staging DMAs are interleaved between per-bank tail transfers so they
    never queue behind a 2.8us group transfer.
"""

import os
import sys
from dataclasses import dataclass

sys.path.insert(0, "/opt/trn_rl_repo")

import numpy as np
import ml_dtypes

import concourse.hw_specs as _hw_specs
from concourse import bass, bacc, mybir, tile
from concourse.bass_utils import run_bass_kernel_spmd

THRESHOLD = 1e-4
EPSILON = 1e-3
Q_SCALE = 16.0  # host pre-scale on rel embedding -> q
T_SCALE = 4.0   # host pre-scale on entity embeddings (fp8 inputs)

f32 = mybir.dt.float32
fp16 = mybir.dt.float16
bf16 = mybir.dt.bfloat16
fp8 = mybir.dt.float8e4
i32 = mybir.dt.int32
u8 = mybir.dt.uint8

# The greedy act-table pass picks, per activation, the first table set
# containing its function; Exp and Ln live in different first-fit sets and
# would force 1.3us table swaps mid-kernel.  Strip Exp/Ln from every set
# except the combined natural_log_exp_and_others (set ids keep their
# act_info.json indices, so walrus still loads the right table).
_orig_get_tables = _hw_specs.get_activation_tables


def _patched_get_tables(arch):
    tabs = _orig_get_tables(arch)
    exp, ln = mybir.ActivationFunctionType.Exp, mybir.ActivationFunctionType.Ln
    return {
        k: (v if k == "natural_log_exp_and_others" else v - {exp, ln})
        for k, v in tabs.items()
    }


_hw_specs.get_activation_tables = _patched_get_tables
bacc.get_activation_tables = _patched_get_tables

# entity-tile groups (start_bank, n_banks): one psum quad is <=4 banks of
# 500 entities; ramped small at the start so PE/Act start early.
GROUPS = [(0, 1), (1, 2), (3, 3), (6, 4), (10, 4), (14, 4), (18, 4), (22, 3)]
EARLY = (0, 1, 2, 3)  # groups that run the pre-bias plain-exp + DVE path
# output chunks (start_bank, end_bank, ready_after_group_index)
OCHUNKS = [(0, 3, 1), (3, 10, 3), (10, 18, 5), (18, 22, 6), (22, 25, 7)]


@dataclass(frozen=True)
class Cfg:
    n_cores: int = 8
    n_ent: int = 100000
    d: int = 512
    h: int = 256
    et: int = 500  # entity tile (psum bank free dim)
    p_pad: int = 512  # padded observed-pair count per core
    s_cols: int = 8  # scatter batches of 128
    hi: float = 1.0 - EPSILON
    do_scatter: bool = True

    @property
    def e_sh(self):
        return self.n_ent // self.n_cores

    @property
    def n_ht(self):
        return self.h // 128

    @property
    def n_k(self):
        return self.d // 128


_compile_cache = {}


def _build(cfg: Cfg, single: bool = False):
    D, H, E_SH, ET = cfg.d, cfg.h, cfg.e_sh, cfg.et
    N_K, N_HT = cfg.n_k, cfg.n_ht
    p_pad, s_cols = cfg.p_pad, cfg.s_cols
    ES = 1.0 / (Q_SCALE * T_SCALE)
    DR = mybir.MatmulPerfMode.DoubleRow
    Exp = mybir.ActivationFunctionType.Exp
    n_ob = p_pad // 512
    assert 1 <= n_ob <= 4
    assert sum(nb for _, nb in GROUPS) * ET == E_SH
    early_cols = sum(nb for gi, (_, nb) in enumerate(GROUPS) if gi in EARLY) * ET

    QW = N_K * H
    OW = N_K * p_pad

    nc = bacc.Bacc(
        "TRN2",
        target_bir_lowering=False,
        debug=False,
        num_devices=1 if single else cfg.n_cores,
    )

    # q + observed tails packed into one DMA (all fp8, [p][k][col] layout
    # with contraction dim d = k*128 + p)
    qt0 = nc.dram_tensor("qt0", [128, QW + OW], fp8, kind="ExternalInput").ap()
    # tails, group-major packed: every group DMA is 128 fat contiguous
    # descriptors
    tailsP = nc.dram_tensor(
        "tailsP", [128, N_K * E_SH], fp8, kind="ExternalInput"
    ).ap()
    a2 = nc.dram_tensor("a2", [H, p_pad], bf16, kind="ExternalInput").ap()
    consts = nc.dram_tensor("consts", [4, 128], f32, kind="ExternalInput").ap()
    if cfg.do_scatter:
        scat = nc.dram_tensor("scat", [s_cols, 128], i32, kind="ExternalInput").ap()
        ones8 = nc.dram_tensor("ones8", [1, 128], u8, kind="ExternalInput").ap()
    out = nc.dram_tensor("out", [H, E_SH], u8, kind="ExternalOutput").ap()

    with tile.TileContext(nc) as tc:
        with (
            tc.tile_pool(name="persist", bufs=1) as pp,
            tc.tile_pool(name="stream", bufs=3) as sp,
            tc.tile_pool(name="psum", bufs=2, space="PSUM") as psp,
            tc.tile_pool(name="dram", bufs=1, space="DRAM") as dp,
        ):
            # ---- q then observed tails (two DMAs: Ldweights can start
            # on q while the obs tails are still in flight) ----
            qt0_sb = pp.tile([128, QW + OW], fp8)
            nc.sync.dma_start(out=qt0_sb[:, :QW], in_=qt0[:, :QW])
            nc.sync.dma_start(out=qt0_sb[:, QW:], in_=qt0[:, QW:])
            q3 = qt0_sb[:, :QW].rearrange("p (k h) -> p k h", k=N_K)
            tobs3 = qt0_sb[:, QW:].rearrange("p (k e) -> p k e", k=N_K)

            # warm the combined Exp/Ln activation table while inputs stream
            warm = pp.tile([128, 1], f32)
            nc.vector.memset(warm[:], 0.0)
            nc.scalar.activation(out=warm[:], in_=warm[:], func=Exp)

            _skip = set(os.environ.get("KSKIP", "").split(","))


            # first three tail groups up front; later groups are issued
            # in completion order.  split=True breaks a group into per-bank
            # DMAs so tiny all-reduce staging DMAs find FIFO holes.
            tt_views = [None] * len(GROUPS)
            tt_tiles = [None] * len(GROUPS)

            def issue_tail(gi, banks=None, split=False):
                g0, nb = GROUPS[gi]
                if tt_tiles[gi] is None:
                    tt_tiles[gi] = sp.tile([128, N_K * 4 * ET], fp8, tag="tt", name=f"tt{gi}")
                    tt_views[gi] = tt_tiles[gi][:, : N_K * nb * ET].rearrange(
                        "p (k e) -> p k e", k=N_K
                    )
                t = tt_tiles[gi]
                rng = range(nb) if banks is None else banks
                if not split:
                    lo, hi_ = min(rng), max(rng) + 1
                    nc.sync.dma_start(
                        out=t[:, N_K * lo * ET : N_K * hi_ * ET],
                        in_=tailsP[:, N_K * (g0 + lo) * ET : N_K * (g0 + hi_) * ET],
                    )
                else:
                    for b in rng:
                        nc.sync.dma_start(
                            out=t[:, N_K * b * ET : N_K * (b + 1) * ET],
                            in_=tailsP[:, N_K * (g0 + b) * ET : N_K * (g0 + b + 1) * ET],
                        )

            # small inputs (scatter inputs are issued at the very end:
            # they are only needed after the last output chunk)
            a2_sb = pp.tile([128, N_HT * p_pad], bf16)
            c_sb = pp.tile([128, 4], f32)
            do_scat = cfg.do_scatter and "scat" not in _skip

            issue_tail(0)
            nc.sync.dma_start(
                out=a2_sb[:].rearrange("p (t e) -> p t e", t=N_HT),
                in_=a2.rearrange("(t p) e -> p t e", t=N_HT),
            )
            nc.sync.dma_start(out=c_sb[:], in_=consts.rearrange("q p -> p q"))
            issue_tail(1)
            issue_tail(2, split=True)

            # ---- observed-pair scores -> eo (also warms the PE) ----
            # both head-tiles share one psum tile so a single act / multiply
            # / reduce covers the whole observed path (it gates the bias)
            eo = pp.tile([128, N_HT * p_pad], bf16)
            pso = psp.tile([128, 4, 512], f32, tag="quad")
            for ht in range(N_HT):
                for nk in range(n_ob):
                    for j in range(N_K // 2):
                        nc.tensor.matmul(
                            out=pso[:, ht * n_ob + nk, :],
                            lhsT=q3[:, 2 * j : 2 * j + 2, ht * 128 : ht * 128 + 128],
                            rhs=tobs3[:, 2 * j : 2 * j + 2, nk * 512 : nk * 512 + 512],
                            start=(j == 0),
                            stop=(j == N_K // 2 - 1),
                            perf_mode=DR,
                        )
            nc.scalar.activation(
                out=eo[:].rearrange("p (b e) -> p b e", b=N_HT * n_ob),
                in_=pso[:, 0 : N_HT * n_ob, :],
                func=Exp,
                scale=ES,
            )

            # ---- D partials per head, pack, all-reduce (1 KB) ----
            zd = pp.tile([128, N_HT], f32)
            scr = pp.tile([128, N_HT * p_pad], bf16)
            nc.vector.tensor_tensor(
                out=scr[:], in0=eo[:], in1=a2_sb[:], op=mybir.AluOpType.mult
            )
            for ht in range(N_HT):
                nc.vector.reduce_sum(
                    out=zd[:, ht : ht + 1],
                    in_=scr[:, ht * p_pad : (ht + 1) * p_pad],
                    axis=mybir.AxisListType.X,
                )
            cc_in = dp.tile([N_HT, 128], f32)
            cc_out = dp.tile([N_HT, 128], f32, addr_space="Shared")
            r_red = pp.tile([128, N_HT], f32)
            def cc_send():
                nc.sync.dma_start(out=cc_in.rearrange("q p -> p q"), in_=zd[:])
                if not single:
                    nc.gpsimd.collective_compute(
                        "AllReduce",
                        mybir.AluOpType.add,
                        replica_groups=[list(range(cfg.n_cores))],
                        ins=[cc_in.opt()],
                        outs=[cc_out.opt()],
                    )

            def read_r_red():
                # cost-model variant: the AllReduce itself is the +12us
                # adder; the boundary write/read DMAs stay charged.  High
                # priority: the scheduler must not hoist later tail-group
                # transfers ahead of this 1KB readback in the DMA FIFO.
                src_cc = cc_in if single else cc_out
                with tc.high_priority():
                    nc.sync.dma_start(
                        out=r_red[:], in_=src_cc.rearrange("q p -> p q")
                    )

            # ---- per-head scale mK = w / (D + nsel), bias b = ln(mK) ----
            # consts cols 0..1 = nsel (keeps the denominator finite for heads
            # with no observations), cols 2..3 = w = sel*cnt*255/hi or 1e-30
            # (so Ln stays finite and exp(bias) rounds to u8 zero).
            t2 = pp.tile([128, N_HT], f32)
            r2 = pp.tile([128, N_HT], f32)
            pw = pp.tile([128, N_HT], f32)
            b2 = pp.tile([128, N_HT], f32)

            def m_chain():
                nc.vector.tensor_tensor(
                    out=t2[:], in0=r_red[:], in1=c_sb[:, 0:N_HT],
                    op=mybir.AluOpType.add,
                )
                nc.vector.reciprocal(out=r2[:], in_=t2[:])
                nc.vector.tensor_tensor(
                    out=pw[:], in0=r2[:], in1=c_sb[:, N_HT : 2 * N_HT],
                    op=mybir.AluOpType.mult,
                )

            # ---- main loop ----
            # (gi, ht) units the idle Pool engine computes via the quadratic
            # exp approx (err <= 1 u8 code): E*mK = mK*(1 + y + y^2/2),
            # y = score*ES.  These sit on psum buf0 so the Act h0 stream on
            # buf1 never waits on Pool.
            POOL_UNITS = set()  # Pool+PSUM tensor ops trip the BIR verifier
            o_big = [pp.tile([128, E_SH], u8, name=f"obig{ht}") for ht in range(N_HT)]
            e_tmp = [
                pp.tile([128, early_cols], fp16, name=f"etmp{ht}")
                for ht in range(N_HT)
            ]
            w1 = pp.tile([128, 4 * ET], f32)
            w2 = pp.tile([128, 4 * ET], fp16)

            def do_group(gi):
                g0, nb = GROUPS[gi]
                tt3 = tt_views[gi]
                for ht in range(N_HT):
                    ps = psp.tile([128, 4, 512], f32, tag="quad")
                    for b in range(nb):
                        for j in range(N_K // 2):
                            nc.tensor.matmul(
                                out=ps[:, b, 0:ET],
                                lhsT=q3[
                                    :, 2 * j : 2 * j + 2, ht * 128 : ht * 128 + 128
                                ],
                                rhs=tt3[:, 2 * j : 2 * j + 2, b * ET : (b + 1) * ET],
                                start=(j == 0),
                                stop=(j == N_K // 2 - 1),
                                perf_mode=DR,
                            )
                    if (gi, ht) in POOL_UNITS:
                        cols = nb * ET
                        w1v = w1[:, :cols].rearrange("p (b e) -> p b e", b=nb)
                        w2v = w2[:, :cols].rearrange("p (b e) -> p b e", b=nb)
                        nc.gpsimd.tensor_scalar(
                            out=w1v,
                            in0=ps[:, 0:nb, 0:ET],
                            scalar1=ES * ES / 2.0,
                            scalar2=ES,
                            op0=mybir.AluOpType.mult,
                            op1=mybir.AluOpType.add,
                        )
                        nc.gpsimd.tensor_tensor(
                            out=w2v,
                            in0=w1v,
                            in1=ps[:, 0:nb, 0:ET],
                            op=mybir.AluOpType.mult,
                        )
                        # scalar-AP tensor_scalar trips the BIR verifier on
                        # Pool; the final (w2*mK + mK) -> u8 rides DVE instead
                        nc.vector.tensor_scalar(
                            out=o_big[ht][:, g0 * ET : (g0 + nb) * ET],
                            in0=w2[:, :cols],
                            scalar1=1.0,
                            scalar2=pw[:, ht : ht + 1],
                            op0=mybir.AluOpType.add,
                            op1=mybir.AluOpType.mult,
                        )
                    elif gi in EARLY:
                        # pre-bias path: plain exp to fp16 staging; the DVE
                        # applies (E * mK) -> u8 once the all-reduce lands
                        nc.scalar.activation(
                            out=e_tmp[ht][:, g0 * ET : (g0 + nb) * ET].rearrange(
                                "p (b e) -> p b e", b=nb
                            ),
                            in_=ps[:, 0:nb, 0:ET],
                            func=Exp,
                            scale=ES,
                        )
                    else:
                        nc.scalar.activation(
                            out=o_big[ht][:, g0 * ET : (g0 + nb) * ET].rearrange(
                                "p (b e) -> p b e", b=nb
                            ),
                            in_=ps[:, 0:nb, 0:ET],
                            func=Exp,
                            scale=ES,
                            bias=b2[:, ht : ht + 1],
                        )

            def convs(gi):
                g0, nb = GROUPS[gi]
                for ht in range(N_HT):
                    nc.vector.tensor_scalar(
                        out=o_big[ht][:, g0 * ET : (g0 + nb) * ET],
                        in0=e_tmp[ht][:, g0 * ET : (g0 + nb) * ET],
                        scalar1=pw[:, ht : ht + 1],
                        scalar2=255.0,
                        op0=mybir.AluOpType.mult,
                        op1=mybir.AluOpType.min,
                    )

            def chunk_out(ci):
                c0b, c1b, _ = OCHUNKS[ci]
                for ht in range(N_HT):
                    nc.sync.dma_start(
                        out=out[ht * 128 : (ht + 1) * 128, c0b * ET : c1b * ET],
                        in_=o_big[ht][:, c0b * ET : c1b * ET],
                    )

            do_group(0)
            do_group(1)
            # g3 goes out per-bank with the two 1KB all-reduce staging DMAs
            # interleaved: their waits are clear when SP reaches them (no
            # head-block) and the FIFO hole between banks takes them early
            issue_tail(3, banks=(0, 1), split=True)
            cc_send()
            issue_tail(3, banks=(2, 3), split=True)
            read_r_red()
            m_chain()
            do_group(2)
            # Ln sits after g2's plain acts on the Act queue; bias is ready
            # well before g3's biased act needs it
            nc.scalar.activation(
                out=b2[:], in_=pw[:], func=mybir.ActivationFunctionType.Ln
            )
            convs(0)
            convs(1)
            convs(2)
            issue_tail(4)
            issue_tail(5)
            chunk_out(0)
            do_group(3)
            convs(3)
            issue_tail(6)
            chunk_out(1)
            do_group(4)
            issue_tail(7)
            do_group(5)
            chunk_out(2)
            do_group(6)
            chunk_out(3)
            do_group(7)
            chunk_out(4)

            # ---- observed positions -> 255 (decodes to exactly 1.0) ----
            if do_scat:
                ones_sb = pp.tile([128, 1], u8)
                nc.sync.dma_start(out=ones_sb[:], in_=ones8.rearrange("q p -> p q"))
                idx_sb = pp.tile([128, s_cols], i32)
                nc.sync.dma_start(out=idx_sb[:], in_=scat.rearrange("s p -> p s"))
                out_flat = out.rearrange("h e -> (h e)")[:, None]
                for j in range(s_cols):
                    nc.gpsimd.indirect_dma_start(
                        out=out_flat,
                        out_offset=bass.IndirectOffsetOnAxis(
                            ap=idx_sb[:, j : j + 1], axis=0
                        ),
                        in_=ones_sb[:],
                        in_offset=None,
                        bounds_check=H * E_SH - 1,
                        oob_is_err=False,
                    )

    nc.compile()
    return nc


def _prepare(cfg_base, ent_emb, rel_emb, head_ent_vec, obs_idx, obs_mask, rel_id,
             num_heads, train_mask):
    """Host-side sharding prep. Returns (cfg, in_maps)."""
    ent_emb = np.asarray(ent_emb, dtype=np.float32)
    rel_emb = np.asarray(rel_emb, dtype=np.float32)
    head_ent_vec = np.asarray(head_ent_vec, dtype=np.float32)
    obs_idx = np.asarray(obs_idx, dtype=np.int32)
    obs_mask = np.asarray(obs_mask, bool)
    rel_id = int(rel_id)
    num_heads = int(num_heads)
    train_mask = int(train_mask)

    D, H = cfg_base.d, cfg_base.h
    E_SH, N_CORES, N_HT = cfg_base.e_sh, cfg_base.n_cores, cfg_base.n_ht
    assert ent_emb.shape == (cfg_base.n_ent, D)
    assert num_heads == H

    heads = np.flatnonzero(head_ent_vec != 0.0)
    assert heads.size == H, f"expected {H} heads, got {heads.size}"

    ent_f8 = (ent_emb * T_SCALE).astype(ml_dtypes.float8_e4m3)
    r = (rel_emb[rel_id] * Q_SCALE).astype(np.float32)
    h_rows = ent_emb[heads]
    # q = complex-mult(h, r) in the transposed [128, k, H] SBUF layout the
    # matmuls consume: row p, block k holds q-matrix dim d = k*128 + p.
    RANK = D // 2
    re_h, im_h = h_rows[:, :RANK], h_rows[:, RANK:]
    re_r, im_r = r[:RANK], r[RANK:]
    q = np.concatenate(
        [re_h * re_r - im_h * im_r, re_h * im_r + im_h * re_r], axis=1
    )  # [H, D]
    n_k = D // 128
    qT8_np = np.ascontiguousarray(
        q.T.reshape(n_k, 128, H).transpose(1, 0, 2).reshape(128, n_k * H)
    ).astype(ml_dtypes.float8_e4m3)

    owner = obs_idx // E_SH
    local = obs_idx - owner * E_SH
    valid = obs_mask
    obs_num = valid.sum(axis=1).astype(np.float32)
    sel = obs_num > 0
    hi = 1.0 - EPSILON if train_mask else 1.0
    kq = 255.0 / hi
    # cols 0..1 add to [D0, D1] (nsel keeps the denominator finite);
    # cols 2..3 are the bias weights w with w=1e-30 for no-obs heads so the
    # Ln stays finite and exp(bias) rounds to u8 zero.
    w = np.where(sel, obs_num * kq, 1e-30).astype(np.float32)
    nsel = (~sel).astype(np.float32)
    consts_np = np.zeros((4, 128), np.float32)
    for ht in range(N_HT):
        sl = slice(ht * 128, (ht + 1) * 128)
        consts_np[ht] = nsel[sl]
        consts_np[N_HT + ht] = w[sl]

    # The scatter must run on the shard owning each (head, tail) position,
    # but the D-partials are all-reduced, so pair SCORING can go to any
    # core: balance pairs evenly to minimize the padded per-core count.
    per_core_scat = []
    for c in range(N_CORES):
        ii, kk = np.nonzero(valid & (owner == c))
        per_core_scat.append((ii, kk))
    max_scat = max(len(ii) for ii, _ in per_core_scat)
    ii_all, kk_all = np.nonzero(valid)
    per_core_bal = [
        (ii_all[sl], kk_all[sl])
        for sl in np.array_split(np.arange(len(ii_all)), N_CORES)
    ]
    max_pairs = max(len(ii) for ii, _ in per_core_bal)
    p_pad = max(512, int(np.ceil(max_pairs / 512.0)) * 512)
    do_scatter = bool(train_mask)
    s_cols = int(np.ceil(max(max_scat, 1) / 128.0)) if do_scatter else 1

    cfg = Cfg(
        n_cores=N_CORES,
        n_ent=cfg_base.n_ent,
        d=D,
        h=H,
        et=cfg_base.et,
        p_pad=p_pad,
        s_cols=s_cols,
        hi=hi,
        do_scatter=do_scatter,
    )
    ET = cfg.et

    in_maps = []
    for c in range(N_CORES):
        ii, kk = per_core_bal[c]
        npair = len(ii)
        g_idx = obs_idx[ii, kk]

        # observed tails packed [p][k][pair] like the main tails
        tpad = np.zeros((p_pad, D), dtype=ml_dtypes.float8_e4m3)
        if npair:
            tpad[:npair] = ent_f8[g_idx]
        tobsP = np.ascontiguousarray(
            tpad.T.reshape(n_k, 128, p_pad).transpose(1, 0, 2).reshape(128, -1)
        )
        a2_np = np.zeros((H, p_pad), ml_dtypes.bfloat16)
        if npair:
            a2_np[ii, np.arange(npair)] = 1.0

        # pack tails group-major: block g = [k][e in group] per partition row
        shard = ent_f8[c * E_SH : (c + 1) * E_SH]  # [E_SH, D]
        t_pke = shard.reshape(E_SH, n_k, 128).transpose(2, 1, 0)
        blocks = [
            t_pke[:, :, g0 * ET : (g0 + nb) * ET].reshape(128, -1)
            for g0, nb in GROUPS
        ]
        im = {
            "qt0": np.ascontiguousarray(np.concatenate([qT8_np, tobsP], axis=1)),
            "tailsP": np.ascontiguousarray(np.concatenate(blocks, axis=1)),
            "a2": a2_np,
            "consts": consts_np,
        }
        if do_scatter:
            si, sk = per_core_scat[c]
            ns = len(si)
            l_idx = local[si, sk]
            scat_np = np.full((s_cols * 128,), 2**30, np.int32)
            if ns:
                scat_np[:ns] = (si.astype(np.int64) * E_SH + l_idx).astype(np.int32)
            im["scat"] = scat_np.reshape(s_cols, 128)
            im["ones8"] = np.full((1, 128), 255, np.uint8)
        in_maps.append(im)

    return cfg, in_maps


def kernel(ent_emb, rel_emb, head_ent_vec, obs_idx, obs_mask, rel_id, num_heads,
           train_mask):
    cfg, in_maps = _prepare(
        Cfg(), ent_emb, rel_emb, head_ent_vec, obs_idx, obs_mask, rel_id,
        num_heads, train_mask,
    )
    if cfg not in _compile_cache:
        _compile_cache[cfg] = _build(cfg)
    nc = _compile_cache[cfg]
    res = run_bass_kernel_spmd(nc, in_maps, core_ids=list(range(cfg.n_cores)))
    out_u8 = np.concatenate(
        [res.results[c]["out"] for c in range(cfg.n_cores)], axis=1
    )
    # u8 decode: code c -> c*hi/255, with 255 -> exactly 1.0 (the clamped-
    # at-hi codes decode to 1.0, inside tolerance; observed scatters exact).
    lut = (np.arange(256) * (cfg.hi / 255.0)).astype(np.float32)
    lut[255] = 1.0
    return lut[out_u8]
